# revision 24
# baseline (speedup 1.0000x reference)
"""BiDAF attention layer on 8 Trainium2 NeuronCores (Bass/Tile).

Math (per batch b):
  t[i,j]  = sum_d (c[i,d]*w_cq[d] + w_q[d]) * q[j,d]   (= cq + sq0[j])
  a       = softmax_j(t)            (biases b_c/b_q/b_cq cancel in softmax)
  c2q     = a @ q
  m[i]    = max_j t[i,j];  sc0[i] = c[i,:]@w_c
  bvec    = softmax_i(m + sc0)      (biases cancel here too)
  q2c     = bvec @ c
  out     = [c | c2q | c*c2q | c*q2c]

Sharding: data-parallel over batch, 4 batches per core, params replicated.

Implementation notes:
  - w_q is folded into the transposed-c operand: chatT = w_cq*cT + w_q,
    applied for free in the PSUM evacuation (tensor_scalar mult+add with
    per-partition vectors). The q@w_q row term then emerges from the score
    contraction itself -- no separate sq0 computation, no exp bias.
  - Score/attention matmuls run in fp16 (fp32 matmul is 2 passes + 2
    LDWEIGHTS on TRN2; 16-bit is 1 pass + FWL), accumulating f32 PSUM.
    c stays f32 end-to-end for the output blocks and products.
  - cT/qT built via PE transposes (contraction over d needs d on
    partitions for both operands). The c PSUM is evacuated twice: affine
    -> chatT (scores) and plain -> cT (for the sc0 matvec).
  - Scores computed twice on PE: once as t [i,j] (row-max for bvec), once
    as tT [j,i] so ScalarE exp() lands e^T in SBUF in exactly the lhsT
    layout the c2q matmul needs (no e-transposes).
  - Softmax skips max-subtraction (|t| <= ~10, exp safe in f32/fp16) and
    the row sum l is fused into the c2q matmul as a ones column of rhs.
  - DMA is split across the three DGE paths to avoid head-of-line
    blocking: c-in/c-out on sync(SP), stage-out on scalar(ACT), q-in
    (with f32->fp16 cast) and c4-out on gpsimd(SWDGE).
"""

import sys

if "/opt/trn_rl_repo" not in sys.path:
    sys.path.insert(0, "/opt/trn_rl_repo")

import numpy as np

import concourse.bass as bass
import concourse.tile as tile
from concourse import bacc, mybir
from concourse.bass import ds, ts
from concourse.masks import make_identity

B, CL, QL, D = 32, 1024, 512, 256
NCORES = 8
BS = B // NCORES  # batches per core
P = 128
F32 = mybir.dt.float32
F16 = mybir.dt.float16

NT = CL // P  # 8 i-tiles
NJ = QL // P  # 4 j-chunks
ND = D // P   # 2 d-chunks
NH = 2        # i-halves for the [j,i]-layout score matmul
IH = CL // NH  # 512
KPH = NT // NH  # i-tiles per half

Exp = mybir.ActivationFunctionType.Exp
AxX = mybir.AxisListType.X
Mult = mybir.AluOpType.mult
Add = mybir.AluOpType.add


def build_bass(bs: int = BS):
    nc = bacc.Bacc(None)
    c_d = nc.declare_dram_parameter("c", [bs, CL, D], F32, isOutput=False)
    q_d = nc.declare_dram_parameter("q", [bs, QL, D], F32, isOutput=False)
    wc_d = nc.declare_dram_parameter("wc_cols", [P, ND], F16, isOutput=False)
    wq_d = nc.declare_dram_parameter("wq_cols", [P, ND], F32, isOutput=False)
    wcq_d = nc.declare_dram_parameter("wcq_cols", [P, ND], F32, isOutput=False)
    out_d = nc.declare_dram_parameter("out", [bs, CL, 4 * D], F32, isOutput=True)

    with tile.TileContext(nc) as tc:
        with (
            tc.tile_pool(name="consts", bufs=1) as consts,
            tc.tile_pool(name="io", bufs=3) as io,
            tc.tile_pool(name="ins", bufs=3) as ins,
            tc.tile_pool(name="work", bufs=3) as work,
            tc.tile_pool(name="ps_t", bufs=2, space="PSUM") as ps_t,
            tc.tile_pool(name="ps_tT", bufs=1, space="PSUM") as ps_tT,
            tc.tile_pool(name="ps_s", bufs=2, space="PSUM") as ps_s,
        ):
            ident_f = consts.tile([P, P], F32)
            ident_h = consts.tile([P, P], F16)
            ones_f = consts.tile([P, P], F32)
            ones_h = consts.tile([1, QL], F16)
            neg_shift = consts.tile([P, 1], F32)
            wc_sb = consts.tile([P, ND], F16)
            wq_sb = consts.tile([P, ND], F32)
            wcq_sb = consts.tile([P, ND], F32)

            def emit_inputs(b):
                # fp16 copy of c (re-read + cast) first on the SWDGE ring:
                # it feeds 16 of the 24 transposes that start each batch
                c_h = ins.tile([P, NT, D], F16, tag="c_h")
                nc.gpsimd.dma_start(
                    out=c_h, in_=c_d[b].rearrange("(t p) d -> p t d", p=P)
                )
                # q loaded once, cast f32 -> fp16 in-flight (SWDGE)
                q_sb = ins.tile([P, NJ, D + 1], F16, tag="q_sb")
                nc.gpsimd.dma_start(
                    out=q_sb[:, :, 0:D],
                    in_=q_d[b].rearrange("(t p) d -> p t d", p=P),
                )
                nc.vector.memset(q_sb[:, :, D : D + 1], 1.0)
                c_sb = ins.tile([P, NT, D], F32, tag="c_sb")
                nc.sync.dma_start(
                    out=c_sb, in_=c_d[b].rearrange("(t p) d -> p t d", p=P)
                )
                # output block 0 is just c; store straight from SBUF (SP ring)
                ov = out_d[b].rearrange("(t p) x -> p t x", p=P)
                nc.sync.dma_start(out=ov[:, :, 0:D], in_=c_sb)
                return c_sb, q_sb, c_h, ov

            pending = [emit_inputs(0)]

            for b in range(bs):
                c_sb, q_sb, c_h, ov = pending.pop(0)

                if b == 0:
                    nc.sync.dma_start(out=wc_sb, in_=wc_d[:])
                    nc.sync.dma_start(out=wq_sb, in_=wq_d[:])
                    nc.sync.dma_start(out=wcq_sb, in_=wcq_d[:])
                    make_identity(nc, ident_h)
                    make_identity(nc, ident_f)
                    nc.vector.memset(ones_f, 1.0)
                    nc.vector.memset(ones_h, 1.0)
                    nc.vector.memset(neg_shift, -2.5)
                else:
                    pass
                # prefetch up to two batches ahead of this batch's gpsimd
                # work so loads aren't FIFO-blocked behind c4 products
                if b == 0:
                    for nb in (1, 2):
                        if nb < bs:
                            pending.append(emit_inputs(nb))
                elif b + 2 < bs:
                    pending.append(emit_inputs(b + 2))

                # ---- transpose c_h -> cT (plain) + chatT (affine) ----
                cT = work.tile([P, ND, CL], F16, tag="cT")
                chatT = work.tile([P, ND, CL], F16, tag="chatT")
                for dc in range(ND):
                    for h in range(NH):
                        pst = ps_t.tile([P, IH], F16, tag="t")
                        for k in range(KPH):
                            it = h * KPH + k
                            nc.tensor.transpose(
                                pst[:, ts(k, P)], c_h[:, it, ts(dc, P)], ident_h
                            )
                        sl = ds(h * IH, IH)
                        nc.vector.tensor_scalar(
                            out=chatT[:, dc, sl],
                            in0=pst,
                            scalar1=wcq_sb[:, dc : dc + 1],
                            scalar2=wq_sb[:, dc : dc + 1],
                            op0=Mult,
                            op1=Add,
                        )
                        nc.scalar.copy(cT[:, dc, sl], pst)

                # ------------- transpose q -> qT (fp16) -------------
                qT = work.tile([P, ND, QL], F16, tag="qT")
                for dc in range(ND):
                    pst = ps_t.tile([P, QL], F16, tag="t")
                    for jc in range(NJ):
                        nc.tensor.transpose(
                            pst[:, ts(jc, P)], q_sb[:, jc, ts(dc, P)], ident_h
                        )
                    if dc == 0:
                        nc.scalar.copy(qT[:, dc], pst)
                    else:
                        nc.vector.tensor_copy(qT[:, dc], pst)

                # ---- sc0 rows: [1, IH] per half via M=1 matmuls (fp16) ----
                sc0_row = work.tile([1, CL], F16, tag="sc0r")
                for h in range(NH):
                    ps_sr = ps_t.tile([1, IH], F32, tag="t")
                    for dc in range(ND):
                        nc.tensor.matmul(
                            ps_sr,
                            wc_sb[:, dc : dc + 1],
                            cT[:, dc, ds(h * IH, IH)],
                            start=(dc == 0),
                            stop=(dc == ND - 1),
                        )
                    if h == 0:
                        nc.scalar.copy(sc0_row[0:1, ds(h * IH, IH)], ps_sr)
                    else:
                        nc.vector.tensor_copy(sc0_row[0:1, ds(h * IH, IH)], ps_sr)

                # ---- phase M: scores, e^T, and row maxes ----
                m_all = work.tile([P, NT], F32, tag="m_all")
                eTs = []
                for h in range(NH):
                    tTq = ps_tT.tile([P, NJ, IH], F32, tag="tTq")
                    for jc in range(NJ):
                        for dc in range(ND):
                            nc.tensor.matmul(
                                tTq[:, jc],
                                qT[:, dc, ts(jc, P)],
                                chatT[:, dc, ds(h * IH, IH)],
                                start=(dc == 0),
                                stop=(dc == ND - 1),
                            )
                    eT = work.tile([P, NJ, IH], F16, tag="eT")
                    eTs.append(eT)
                    for jc in range(NJ):
                        nc.scalar.activation(eT[:, jc], tTq[:, jc], Exp)

                    for k in range(KPH):
                        it = h * KPH + k
                        pt = ps_t.tile([P, QL], F32, tag="t")
                        for dc in range(ND):
                            nc.tensor.matmul(
                                pt,
                                chatT[:, dc, ts(it, P)],
                                qT[:, dc],
                                start=(dc == 0),
                                stop=False,
                            )
                        # + sc0[i] broadcast over j (K=1): m_all = max_j t + sc0
                        nc.tensor.matmul(
                            pt,
                            sc0_row[0:1, ts(it, P)],
                            ones_h,
                            start=False,
                            stop=True,
                        )
                        nc.vector.reduce_max(m_all[:, it : it + 1], pt, AxX)

                # ---- bvec numerators (ebv in fp16, shifted by -2.5) ----
                ebv_h = work.tile([P, NT], F16, tag="ebvh")
                nc.scalar.activation(ebv_h, m_all, Exp, bias=neg_shift[:, 0:1])
                colsum = work.tile([P, 1], F32, tag="colsum")
                nc.vector.reduce_sum(colsum, ebv_h, AxX)

                # ---- phase 2a: c2q matmuls for first half ----
                stage = io.tile([P, NT, 2 * D], F32, tag="stage")

                def mm2_tile(h, k):
                    it = h * KPH + k
                    po = ps_s.tile([P, D + 1], F32, tag="s")
                    for jc in range(NJ):
                        nc.tensor.matmul(
                            po,
                            eTs[h][:, jc, ts(k, P)],
                            q_sb[:, jc],
                            start=(jc == 0),
                            stop=(jc == NJ - 1),
                        )
                    linv = work.tile([P, 1], F32, tag="linv")
                    nc.vector.reciprocal(linv, po[:, D : D + 1])
                    nc.scalar.mul(stage[:, it, 0:D], po[:, 0:D], linv)
                    nc.vector.tensor_mul(
                        stage[:, it, D : 2 * D],
                        c_sb[:, it],
                        stage[:, it, 0:D],
                    )

                for k in range(KPH):
                    mm2_tile(0, k)
                nc.scalar.dma_start(
                    out=ov[:, 0:KPH, D : 3 * D], in_=stage[:, 0:KPH]
                )

                # ---- q2c chain (PE parts emitted after mm2 h0 so the
                # colsum/ebv dependencies are already satisfied) ----
                ps_tot = ps_s.tile([P, 1], F32, tag="s")
                nc.tensor.matmul(ps_tot, ones_f, colsum, start=True, stop=True)
                totinv = work.tile([P, 1], F32, tag="totinv")
                nc.vector.reciprocal(totinv, ps_tot)
                ps_q2c = ps_s.tile([1, D], F32, tag="s")
                for it in range(NT):
                    nc.tensor.matmul(
                        ps_q2c,
                        ebv_h[:, it : it + 1],
                        c_h[:, it],
                        start=(it == 0),
                        stop=(it == NT - 1),
                    )
                q2c_row = work.tile([1, D], F32, tag="q2cr")
                nc.vector.tensor_scalar_mul(q2c_row, ps_q2c, totinv[0:1, 0:1])
                ps_q2cb = ps_t.tile([P, D], F32, tag="t")
                nc.tensor.matmul(
                    ps_q2cb, ones_f[0:1, :], q2c_row, start=True, stop=True
                )
                q2c_sb = work.tile([P, D], F32, tag="q2csb")
                nc.scalar.copy(q2c_sb, ps_q2cb)
                c4st = io.tile([P, NT, D], F32, tag="c4st")
                for it in range(NT):
                    nc.gpsimd.tensor_mul(c4st[:, it], c_sb[:, it], q2c_sb)
                nc.gpsimd.dma_start(out=ov[:, :, 3 * D : 4 * D], in_=c4st)

                # ---- phase 2b: second half ----
                for k in range(KPH):
                    mm2_tile(1, k)
                nc.scalar.dma_start(
                    out=ov[:, KPH:NT, D : 3 * D], in_=stage[:, KPH:NT]
                )

    nc.compile()
    return nc


_NC_CACHE = {}


def _get_nc(bs: int = BS):
    if bs not in _NC_CACHE:
        _NC_CACHE[bs] = build_bass(bs)
    return _NC_CACHE[bs]


def _param_maps(w_c, w_q, w_cq):
    wc_cols = np.ascontiguousarray(
        np.asarray(w_c, np.float32).reshape(ND, P).T.astype(np.float16)
    )
    wq_cols = np.ascontiguousarray(np.asarray(w_q, np.float32).reshape(ND, P).T)
    wcq_cols = np.ascontiguousarray(
        np.asarray(w_cq, np.float32).reshape(ND, P).T
    )
    return wc_cols, wq_cols, wcq_cols


def _run(c, q, w_c, w_q, w_cq, trace=False, **trace_kwargs):
    from concourse.bass_utils import run_bass_kernel_spmd

    c = np.asarray(c, np.float32)
    q = np.asarray(q, np.float32)
    wc_cols, wq_cols, wcq_cols = _param_maps(w_c, w_q, w_cq)

    nc = _get_nc(BS)
    in_maps = []
    for k in range(NCORES):
        in_maps.append(
            {
                "c": np.ascontiguousarray(c[k * BS : (k + 1) * BS]),
                "q": np.ascontiguousarray(q[k * BS : (k + 1) * BS]),
                "wc_cols": wc_cols,
                "wq_cols": wq_cols,
                "wcq_cols": wcq_cols,
            }
        )
    res = None
    last_err = None
    for attempt in range(3):
        try:
            res = run_bass_kernel_spmd(
                nc,
                in_maps,
                core_ids=list(range(NCORES)),
                trace=trace,
                **trace_kwargs,
            )
            break
        except Exception as e:  # transient device wedges clear on retry
            last_err = e
            if "UNRECOVERABLE" not in str(e) and "UNAVAILABLE" not in str(e):
                raise
    if res is None:
        raise last_err
    out = np.concatenate([res.results[k]["out"] for k in range(NCORES)], axis=0)
    return out, res


def kernel(c, q, w_c, b_c, w_q, b_q, w_cq, b_cq):
    # b_c/b_q/b_cq provably cancel in both softmaxes; output doesn't use them.
    out, _ = _run(c, q, w_c, w_q, w_cq)
    return out
